# revision 6
# baseline (speedup 1.0000x reference)
# Distributed GNN message-passing kernel for one TRN2 chip (8 NeuronCores).
#
# Reference model: 2x SAGEConv(mean) + 1x GraphConv(sum) + linear head +
# softmax over a width-1 axis. N=50000 nodes, E=800000 edges, D=H=64.
#
# Strategy (graph/data parallel, per the sharding hint):
#  - Nodes are sharded contiguously across the 8 cores (6250/core, padded to
#    6272 = 49 blocks of 128). Edges are assigned to the core owning their
#    destination and destination-sorted.
#  - Everything is bf16 except PSUM accumulation and the final head output.
#  - Activations live feature-major in SBUF: hT [65, 6272] bf16 (row 64 = ones
#    so the bias rides inside the self-term matmul).
#  - Per layer: t = h @ Wl.T is computed per-shard (PE), DMAed to DRAM bf16
#    and AllGathered so every core holds t for all nodes (the halo exchange).
#  - Aggregation: edges are processed in 128-edge tiles. dma_gather requires
#    256B elements, so the all-gathered t (bf16, 128B rows) is viewed as
#    [NAG/2, 128] node PAIRS; the gather index is the pair row (< 25088, so a
#    single int16 index space — no lo/hi split). Each tile's edges share a
#    src-row parity chosen at preprocessing time, so the scatter matmul reads
#    lhsT = g[:, t, 0:64] (even) or g[:, t, 64:128] (odd).
#  - The scatter-add is a PE matmul: psum[64f,128n] += msg.T @ M where
#    M[e, j] = (dstloc[e] == j) * scale[e] is built in one DVE tensor_scalar
#    (is_equal, mult) against a resident bf16 iota tile. scale = 1/deg for the
#    mean layers, 1 (0 for padding) for the sum layer.
#  - Gathers round-robin across 4 SWDGE queues; tiles within a block are
#    src-sorted for HBM locality.
#  - The self term (h @ Wr.T + b) accumulates into the same PSUM tile with
#    lhsT = [Wr.T; b] and rhs = hT[0:65, block]. ReLU (ScalarE) writes the
#    next layer's hT directly, so no transposes exist anywhere.
#  - Head: logits = h3 @ Wlin.T + blin per block; softmax over the width-1
#    axis is exp(x - max)/sum = exp(0)/1, computed as Exp(scale=0).
import functools
import numpy as np

N = 50000
E = 800000
D = 64
NCORES = 8
SHARD = N // NCORES              # 6250
BLK = 128
NBLK = (SHARD + BLK - 1) // BLK  # 49
SHARD_PAD = NBLK * BLK           # 6272
NAG = NCORES * SHARD_PAD         # 50176
CHUNK_BLKS = 7                   # blocks per gather chunk (49 = 7*7)
GATHER_TILES = 8                 # max 128-edge tiles per dma_gather call
                                 # (hardware wedges somewhere in (1024, 2048]
                                 # indices per call; 8*128 = 1024 is validated)
NQUEUES = 4                      # SWDGE queues for gather round-robin

_DEBUG_H3 = False  # when True, kernel also returns per-core hT3 [64, SHARD_PAD]


def _agrow(src):
    # row of node `src` inside the all-gathered tensor [NCORES*SHARD_PAD, 64]
    return (src // SHARD) * SHARD_PAD + (src % SHARD)


def _preprocess(edge_index):
    """Host-side index preprocessing. Returns per-core arrays + the shared
    static tile structure (identical across cores so the SPMD program is
    uniform). Per block, edges are split by src-row parity (even tiles then
    odd tiles), each group sorted by src for gather locality."""
    src = np.asarray(edge_index[0], dtype=np.int64)
    dst = np.asarray(edge_index[1], dtype=np.int64)
    deg = np.bincount(dst, minlength=N).astype(np.float64)
    invdeg = (1.0 / np.maximum(deg, 1.0)).astype(np.float32)

    rows = _agrow(src)
    core_of = dst // SHARD

    # per (core, block, parity) edge lists
    per_core = []
    for k in range(NCORES):
        m = core_of == k
        r_k = rows[m]
        d_k = dst[m] - k * SHARD
        blk_k = d_k >> 7
        lists = []
        for b in range(NBLK):
            mb = blk_k == b
            r_b = r_k[mb]
            d_b = d_k[mb] - b * BLK
            par = (r_b & 1).astype(bool)
            out = []
            for sel in (~par, par):
                r_s = r_b[sel]
                d_s = d_b[sel]
                order = np.argsort(r_s, kind="stable")  # src-sorted
                out.append((r_s[order] >> 1, d_s[order]))
            lists.append(out)
        per_core.append(lists)

    # uniform tile counts per (block, parity): max over cores
    nte = np.zeros(NBLK, dtype=np.int64)
    nto = np.zeros(NBLK, dtype=np.int64)
    for b in range(NBLK):
        for k in range(NCORES):
            (re_, _), (ro_, _) = per_core[k][b]
            nte[b] = max(nte[b], (len(re_) + BLK - 1) // BLK)
            nto[b] = max(nto[b], (len(ro_) + BLK - 1) // BLK)
        nte[b] = max(nte[b], 1)
        nto[b] = max(nto[b], 1)

    T = int(nte.sum() + nto.sum())
    # tile offset of block b = sum over previous blocks of (nte+nto)
    blk_tiles = nte + nto
    tile_off = np.concatenate([[0], np.cumsum(blk_tiles)]).astype(int)

    def build_core(k):
        idx = np.zeros(T * BLK, dtype=np.int16)
        dl = np.zeros(T * BLK, dtype=np.float32)
        scm = np.zeros(T * BLK, dtype=np.float32)
        sc1 = np.zeros(T * BLK, dtype=np.float32)
        for b in range(NBLK):
            off = int(tile_off[b]) * BLK
            for p, (r_s, d_s) in enumerate(per_core[k][b]):
                n = len(r_s)
                idx[off : off + n] = r_s
                dl[off : off + n] = d_s
                scm[off : off + n] = invdeg[d_s + b * BLK + k * SHARD]
                sc1[off : off + n] = 1.0
                off += int(nte[b] if p == 0 else nto[b]) * BLK

        def wrap_idx(a):
            # idx j of a tile lives at [j%16, j//16]; replicate 16 rows to 128
            w = a.reshape(-1, 16).T  # [16, cols]
            return np.ascontiguousarray(np.tile(w, (8, 1)))  # [128, cols]

        def col_mat(a):
            # edge j of tile t at [j%128, t]
            return np.ascontiguousarray(a.reshape(T, BLK).T)

        return {
            "idx": wrap_idx(idx),
            "dl": col_mat(dl),
            "scm": col_mat(scm),
            "sc1": col_mat(sc1),
        }

    cores = [build_core(k) for k in range(NCORES)]
    return cores, tuple(int(x) for x in nte), tuple(int(x) for x in nto)


@functools.lru_cache(maxsize=8)
def _compile(nte, nto, debug_h3, repeat=1, variant=""):
    # variant: comma-set of {"nocc", "nom", "nogather", "seqgather", "q1"} —
    # timing-only ablations
    import concourse.bass as bass
    import concourse.mybir as mybir
    from concourse import bacc, tile

    dt = mybir.dt
    nte = list(nte)
    nto = list(nto)
    T = sum(nte) + sum(nto)
    blk_tiles = [e + o for e, o in zip(nte, nto)]
    tile_off = np.concatenate([[0], np.cumsum(blk_tiles)]).astype(int)
    nq = 1 if "q1" in variant else NQUEUES

    nc = bacc.Bacc(
        "TRN2", target_bir_lowering=False, num_devices=NCORES, num_swdge_queues=nq
    )

    # ---- DRAM parameters -------------------------------------------------
    xT_d = nc.dram_tensor("xT", [D, SHARD_PAD], dt.bfloat16, kind="ExternalInput")
    idx_d = nc.dram_tensor("idx", [128, T * 8], dt.int16, kind="ExternalInput")
    dl_d = nc.dram_tensor("dl", [128, T], dt.float32, kind="ExternalInput")
    scm_d = nc.dram_tensor("scm", [128, T], dt.float32, kind="ExternalInput")
    sc1_d = nc.dram_tensor("sc1", [128, T], dt.float32, kind="ExternalInput")
    iota_d = nc.dram_tensor("iota", [128, 128], dt.bfloat16, kind="ExternalInput")
    wl_d = nc.dram_tensor("wl", [64, 3 * 64], dt.bfloat16, kind="ExternalInput")
    wra_d = nc.dram_tensor("wra", [65, 3 * 64], dt.bfloat16, kind="ExternalInput")
    whead_d = nc.dram_tensor("whead", [65, 1], dt.bfloat16, kind="ExternalInput")
    out_d = nc.dram_tensor("out", [SHARD_PAD, 1], dt.float32, kind="ExternalOutput")
    if debug_h3:
        hdbg_d = nc.dram_tensor(
            "hdbg", [64, SHARD_PAD], dt.bfloat16, kind="ExternalOutput"
        )

    # internal DRAM
    t_loc = nc.dram_tensor("t_loc", [SHARD_PAD, D], dt.bfloat16)
    t_ags = [
        nc.dram_tensor(f"t_ag{l}", [NAG, D], dt.bfloat16, addr_space="Shared")
        for l in range(3 * repeat)
    ]

    # chunk structure
    chunk_blocks = [
        list(range(c, min(c + CHUNK_BLKS, NBLK))) for c in range(0, NBLK, CHUNK_BLKS)
    ]
    max_t_chunk = max(sum(blk_tiles[b] for b in cb) for cb in chunk_blocks)

    from contextlib import ExitStack

    with tile.TileContext(nc) as tc, ExitStack() as ctx:
        pool_const = ctx.enter_context(tc.tile_pool(name="const", bufs=1))
        pool_h = ctx.enter_context(tc.tile_pool(name="hstate", bufs=1))
        pool_g = ctx.enter_context(tc.tile_pool(name="gather", bufs=2))
        # deep M pool: DVE must be able to build a whole chunk's one-hot tiles
        # ahead while the PE waits on the gather DMA, else every scatter
        # matmul eats the ~400ns M-build latency serially.
        pool_m = ctx.enter_context(tc.tile_pool(name="onehot", bufs=64))
        pool_ps_agg = ctx.enter_context(tc.tile_pool(name="psagg", bufs=4, space="PSUM"))
        pool_ps_misc = ctx.enter_context(
            tc.tile_pool(name="psmisc", bufs=2, space="PSUM")
        )

        # ---- resident constants -----------------------------------------
        def load_const(name, dram, shape, dtype):
            t = pool_const.tile(shape, dtype, tag=name, name=name)
            nc.sync.dma_start(t[:], dram.ap())
            return t

        iota_sb = load_const("iota", iota_d, [128, 128], dt.bfloat16)
        wl_sb = load_const("wl", wl_d, [64, 3 * 64], dt.bfloat16)
        wra_sb = load_const("wra", wra_d, [65, 3 * 64], dt.bfloat16)
        whead_sb = load_const("whead", whead_d, [65, 1], dt.bfloat16)
        idx_sb = load_const("idx", idx_d, [128, T * 8], dt.int16)
        dl_sb = load_const("dl", dl_d, [128, T], dt.float32)
        scm_sb = load_const("scm", scm_d, [128, T], dt.float32)
        sc1_sb = load_const("sc1", sc1_d, [128, T], dt.float32)

        # ---- activation state (feature-major, ones row at 64) -----------
        hT = [
            pool_h.tile([65, SHARD_PAD], dt.bfloat16, tag=f"hT{i}", name=f"hT{i}")
            for i in range(2)
        ]
        nc.vector.memset(hT[0][64:65, :], 1.0)
        nc.vector.memset(hT[1][64:65, :], 1.0)

        t_stage = pool_h.tile([128, NBLK * 64], dt.bfloat16, tag="tstage")
        out_stage = pool_h.tile([128, NBLK], dt.float32, tag="ostage")

        t_loc_v = t_loc.ap().rearrange("(b p) f -> p b f", p=128)

        qn = [0]

        def next_q():
            q = qn[0]
            qn[0] = (q + 1) % nq
            return q

        rep_layers = [(rep, l) for rep in range(repeat) for l in range(3)]
        for rep, l in rep_layers:
            if l == 0:
                nc.sync.dma_start(hT[0][0:64, :], xT_d.ap())
            h_cur = hT[l % 2]
            h_nxt = hT[(l + 1) % 2]
            sc_sb = scm_sb if l < 2 else sc1_sb

            # t = h @ Wl.T (node-major), staged then one DMA + AllGather
            for b in range(NBLK):
                ps = pool_ps_misc.tile([128, 64], dt.float32, tag="pst")
                nc.tensor.matmul(
                    ps[:],
                    lhsT=h_cur[0:64, b * BLK : (b + 1) * BLK],
                    rhs=wl_sb[:, l * 64 : (l + 1) * 64],
                    start=True,
                    stop=True,
                )
                nc.scalar.copy(t_stage[:, b * 64 : (b + 1) * 64], ps[:])
            nc.sync.dma_start(
                t_loc_v, t_stage[:].rearrange("p (b f) -> p b f", f=64)
            )
            if "nocc" not in variant:
                nc.gpsimd.collective_compute(
                    "AllGather",
                    mybir.AluOpType.bypass,
                    replica_groups=[list(range(NCORES))],
                    ins=[t_loc.ap()],
                    outs=[t_ags[3 * rep + l].ap()],
                )
            # pair view for 256B-element gathers: [NAG/2, 128]
            t_ag_pairs = (
                t_ags[3 * rep + l].ap().rearrange("(a two) f -> a (two f)", two=2)
            )

            # aggregation over chunks of blocks
            for cb in chunk_blocks:
                c0 = int(tile_off[cb[0]])
                n_t = sum(blk_tiles[b] for b in cb)
                g_t = pool_g.tile([128, max_t_chunk, 128], dt.bfloat16, tag="g")
                if "nogather" in variant:
                    nc.vector.memset(g_t[:, 0:n_t, :], 0.0)
                elif "seqgather" in variant:
                    # same byte volume as the gather, but sequential reads
                    nc.sync.dma_start(
                        g_t[:, 0:n_t, :],
                        t_ag_pairs[0 : n_t * 128, :].rearrange(
                            "(p t) f -> p t f", p=128
                        ),
                    )
                else:
                    for j0 in range(0, n_t, GATHER_TILES):
                        j1 = min(j0 + GATHER_TILES, n_t)
                        nc.gpsimd.dma_gather(
                            g_t[:, j0:j1, :],
                            t_ag_pairs,
                            idx_sb[:, (c0 + j0) * 8 : (c0 + j1) * 8],
                            num_idxs=(j1 - j0) * BLK,
                            num_idxs_reg=(j1 - j0) * BLK,
                            elem_size=128,
                            queue_num=next_q(),
                        )
                for b in cb:
                    noagg = "noagg" in variant
                    ps = pool_ps_agg.tile([64, 128], dt.float32, tag="psagg")
                    # self term + bias: [Wr.T; b].T @ hT[0:65, blk]
                    nc.tensor.matmul(
                        ps[:],
                        lhsT=wra_sb[:, l * 64 : (l + 1) * 64],
                        rhs=h_cur[:, b * BLK : (b + 1) * BLK],
                        start=True,
                        stop=noagg,
                    )
                    n_parts = blk_tiles[b]
                    t0 = int(tile_off[b])
                    for ti in range(n_parts if not noagg else 0):
                        g = t0 + ti  # global tile index
                        tc_i = g - c0  # tile index within the chunk
                        par = 0 if ti < nte[b] else 1
                        if "nom" in variant:
                            m_ap = iota_sb[:]
                        else:
                            m = pool_m.tile([128, 128], dt.bfloat16, tag="m")
                            nc.vector.tensor_scalar(
                                m[:],
                                iota_sb[:],
                                dl_sb[:, g : g + 1],
                                sc_sb[:, g : g + 1],
                                mybir.AluOpType.is_equal,
                                mybir.AluOpType.mult,
                            )
                            m_ap = m[:]
                        nc.tensor.matmul(
                            ps[:],
                            lhsT=g_t[:, tc_i, 64 * par : 64 * par + 64],
                            rhs=m_ap,
                            start=False,
                            stop=(ti == n_parts - 1),
                        )
                    # ReLU -> next layer's feature-major state
                    nc.scalar.activation(
                        h_nxt[0:64, b * BLK : (b + 1) * BLK],
                        ps[:],
                        mybir.ActivationFunctionType.Relu,
                    )

            if l == 2:
                # ---- head: softmax over width-1 axis == exp(0)/1 --------
                h_fin = h_nxt
                for b in range(NBLK):
                    ps = pool_ps_misc.tile([128, 1], dt.float32, tag="pst")
                    nc.tensor.matmul(
                        ps[:],
                        lhsT=h_fin[:, b * BLK : (b + 1) * BLK],
                        rhs=whead_sb[:],
                        start=True,
                        stop=True,
                    )
                    nc.scalar.activation(
                        out_stage[:, b : b + 1],
                        ps[:],
                        mybir.ActivationFunctionType.Exp,
                        scale=0.0,
                    )
                nc.sync.dma_start(
                    out_d.ap().rearrange("(b p) one -> p (b one)", p=128),
                    out_stage[:],
                )
                if debug_h3:
                    nc.sync.dma_start(hdbg_d.ap(), h_fin[0:64, :])

    nc.compile()
    return nc


def _pack_weights(Wl1, Wr1, b1, Wl2, Wr2, b2, Wrel3, Wroot3, b3, Wlin, blin):
    import ml_dtypes

    bf16 = ml_dtypes.bfloat16
    wl = np.concatenate(
        [np.ascontiguousarray(W.T) for W in (Wl1, Wl2, Wrel3)], axis=1
    ).astype(bf16)  # [64, 192]
    wra = np.concatenate(
        [
            np.concatenate([W.T, b[None, :]], axis=0)
            for W, b in ((Wr1, b1), (Wr2, b2), (Wroot3, b3))
        ],
        axis=1,
    ).astype(bf16)  # [65, 192]
    whead = np.concatenate([Wlin.T, blin[None, :]], axis=0).astype(bf16)
    return wl, wra, whead


def build_in_maps(
    x,
    Wl1,
    Wr1,
    b1,
    Wl2,
    Wr2,
    b2,
    Wrel3,
    Wroot3,
    b3,
    Wlin,
    blin,
    edge_index,
):
    import ml_dtypes

    bf16 = ml_dtypes.bfloat16
    x = np.asarray(x, dtype=np.float32)
    edge_index = np.asarray(edge_index)
    cores, nte, nto = _preprocess(edge_index)

    wl, wra, whead = _pack_weights(
        np.asarray(Wl1, np.float32),
        np.asarray(Wr1, np.float32),
        np.asarray(b1, np.float32),
        np.asarray(Wl2, np.float32),
        np.asarray(Wr2, np.float32),
        np.asarray(b2, np.float32),
        np.asarray(Wrel3, np.float32),
        np.asarray(Wroot3, np.float32),
        np.asarray(b3, np.float32),
        np.asarray(Wlin, np.float32),
        np.asarray(blin, np.float32),
    )
    iota = (
        np.broadcast_to(np.arange(128, dtype=np.float32), (128, 128))
        .astype(bf16)
        .copy()
    )

    in_maps = []
    for k in range(NCORES):
        xT = np.zeros((D, SHARD_PAD), dtype=bf16)
        xT[:, :SHARD] = x[k * SHARD : (k + 1) * SHARD].T.astype(bf16)
        m = dict(cores[k])
        m.update(
            xT=np.ascontiguousarray(xT),
            iota=iota,
            wl=wl,
            wra=wra,
            whead=whead,
        )
        in_maps.append(m)
    return in_maps, nte, nto


def kernel(
    x,
    Wl1,
    Wr1,
    b1,
    Wl2,
    Wr2,
    b2,
    Wrel3,
    Wroot3,
    b3,
    Wlin,
    blin,
    edge_index,
):
    from concourse.bass_utils import run_bass_kernel_spmd

    in_maps, nte, nto = build_in_maps(
        x, Wl1, Wr1, b1, Wl2, Wr2, b2, Wrel3, Wroot3, b3, Wlin, blin, edge_index
    )
    nc = _compile(nte, nto, _DEBUG_H3)

    res = run_bass_kernel_spmd(nc, in_maps, list(range(NCORES)))
    out = np.empty((N, 1), dtype=np.float32)
    for k in range(NCORES):
        out[k * SHARD : (k + 1) * SHARD] = res.results[k]["out"][:SHARD]
    kernel._res = res
    if _DEBUG_H3:
        kernel._h3 = np.concatenate(
            [
                res.results[k]["hdbg"][:, :SHARD].T.astype(np.float32)
                for k in range(NCORES)
            ],
            axis=0,
        )
    return out
